# revision 14
# baseline (speedup 1.0000x reference)
"""Bidirectional LSTM Trainium2 Bass kernel (transposed formulation).

Problem: T=128, B=128, IN=512, H=512, OUT=512 (fp32 reference).
Sharding: data-parallel over batch + direction-parallel:
  cores 0-3: forward LSTM, batch slices 0:32, 32:64, 64:96, 96:128
  cores 4-7: backward LSTM (time-reversed x), same batch slices

Transposed layout: gates live on the PARTITION axis (16 stationary
chunks of 128 gates = (type o/f/i/g) x (hidden 128-chunk)), the batch
(32) is the matmul moving dim.  PE matmul cost scales with the moving
free size only, so this quarters TensorE work vs. batch-in-partition:
  - phase 1 (xw = x @ W_ih.T + bias) accumulates straight into the
    per-step PSUM bank: bias seeded by a K=1 ones-matmul (start=True),
    then 4 x-k-tile matmuls, then at step time 4 W_hh k-tile matmuls.
  - per-step PSUM bank: tile [128, 4(type), 4(hid chunk), 32] fp32
    (= 512 cols = exactly one 2KB PSUM bank), 6 banks in flight.
  - h is produced directly in transposed layout [hid-in-chunk(128),
    chunk, t, batch] -> no transpose instructions at all; the next
    step's matmuls and phase 3 read it as the moving operand.
  - activations: sigmoid over (o,f,i) block slices, tanh over g;
    cell update on DVE; h = o*tanh(c) on GPSIMD (idle engine) so the
    DVE queue is never blocked behind ACT.
  - phase 3 (out = h @ W_lin_dir.T) per 4-step granule into 1 PSUM
    bank (double buffered), evacuated by GPSIMD, DMA'd per granule.
Host combines: out = out_fwd + flip_t(out_bwd) + b_lin.

All matmuls bf16 (fp32 PSUM accumulation); cell state stays fp32.
"""

import sys

sys.path.insert(0, "/opt/trn_rl_repo")

import functools
import os

import ml_dtypes
import numpy as np

import concourse.bass as bass
import concourse.tile as tile
from concourse import bacc, mybir
from concourse.bass_utils import run_bass_kernel_spmd

T, B, IN, H, OUT = 128, 128, 512, 512, 512
NCORES = 8
BL = B // 4  # batch per core (4 cores per direction)
G4 = 4 * H  # 2048 gate columns
KT = IN // 128  # 4 k-tiles of 128
NCH = 16  # gate M-chunks: (type o/f/i/g) x (hidden chunk 0..3)
OCH = OUT // 128  # 4 output column chunks
TCH = T // 4  # 32 output granules of 4 timesteps

KNOB_LOOKAHEAD = int(os.environ.get("LSTM_LOOKAHEAD", "4"))
KNOB_PG_BUFS = int(os.environ.get("LSTM_PG_BUFS", "3"))

BF16 = mybir.dt.bfloat16
FP32 = mybir.dt.float32
AF = mybir.ActivationFunctionType

LABELS = {}  # instruction name -> human label (for sim diagnostics)


def _lab(inst, label):
    try:
        LABELS[inst.ins.name] = label
    except AttributeError:
        pass
    return inst


def build_nc(reps=1):
    nc = bacc.Bacc(None, target_bir_lowering=False)
    xT = nc.dram_tensor("xT", [128, KT, T, BL], BF16, kind="ExternalInput")
    wih = nc.dram_tensor("wih", [128, KT, G4], BF16, kind="ExternalInput")
    whh = nc.dram_tensor("whh", [128, KT, G4], BF16, kind="ExternalInput")
    bias1 = nc.dram_tensor("bias1", [1, G4], BF16, kind="ExternalInput")
    ones1 = nc.dram_tensor("ones1", [1, 2 * BL], BF16, kind="ExternalInput")
    wlin = nc.dram_tensor("wlin", [128, KT, OUT], BF16, kind="ExternalInput")
    outp = nc.dram_tensor("outp", [128, OCH, T, BL], FP32, kind="ExternalOutput")

    LA = KNOB_LOOKAHEAD

    with tile.TileContext(nc) as tc:
        with (
            tc.tile_pool(name="const", bufs=1) as constp,
            tc.tile_pool(name="acts", bufs=2) as acts_p,
            tc.tile_pool(name="tmps", bufs=2) as tmps_p,
            tc.tile_pool(name="stag", bufs=2) as stag_p,
            tc.tile_pool(name="pg", bufs=KNOB_PG_BUFS, space="PSUM") as pg_p,
            tc.tile_pool(name="ps3", bufs=2, space="PSUM") as ps3_p,
        ):
            wih_sb = constp.tile([128, KT, G4], BF16)
            nc.sync.dma_start(wih_sb[:], wih[:])
            bias_sb = constp.tile([1, G4], BF16)
            nc.sync.dma_start(bias_sb[:], bias1[:])
            ones_sb = constp.tile([1, 2 * BL], BF16)
            nc.sync.dma_start(ones_sb[:], ones1[:])
            # x in 4 time-quarters so phase 1 can start after the first
            x_sb = constp.tile([128, KT, T, BL], BF16)
            for q in range(4):
                nc.sync.dma_start(
                    x_sb[:, :, 32 * q : 32 * q + 32, :], xT[:, :, 32 * q : 32 * q + 32, :]
                )
            whh_sb = constp.tile([128, KT, G4], BF16)
            nc.sync.dma_start(whh_sb[:], whh[:])
            wlin_sb = constp.tile([128, KT, OUT], BF16)
            nc.sync.dma_start(wlin_sb[:], wlin[:])

            # h history, transposed: hh?[p, c, t+1, b] = h_t[128*(2?+c)+p, b]
            # split into per-half tiles so step t+1's k=0,1 matmuls depend
            # only on the A-half write.
            hhA = constp.tile([128, 2, T + 1, BL], BF16)
            hhB = constp.tile([128, 2, T + 1, BL], BF16)
            # cell state [p, hid chunk, b], fp32
            c_st = constp.tile([128, KT, BL], FP32)

            for _rep in range(reps):
                nc.vector.memset(c_st[:], 0.0)
                nc.vector.memset(hhA[:, :, 0, :], 0.0)
                nc.vector.memset(hhB[:, :, 0, :], 0.0)

                pg_tiles = {}

                def emit_phase1(g, khalf):
                    # 2-step granule, half the k-tiles per call: emitted as a
                    # ready-work cushion in front of each step's W_hh block so
                    # the PE exec queue never starves during the h(t-1) wait.
                    # bias seed: K=1 ones matmul (start=True) in the first half.
                    if khalf == 0:
                        pg = pg_p.tile(
                            [128, 4, KT, 2, BL], FP32, tag="pg", name=f"pg{g}"
                        )
                        pg_tiles[g] = pg
                        for m in range(NCH):
                            ty, hc = m // 4, m % 4
                            _lab(nc.tensor.matmul(
                                pg[:, ty, hc],
                                bias_sb[:, 128 * m : 128 * m + 128],
                                ones_sb[:],
                                start=True,
                                stop=False,
                            ), f"p1bias g{g} m{m}")
                    pg = pg_tiles[g]
                    for k in (khalf * 2, khalf * 2 + 1):
                        for m in range(NCH):
                            ty, hc = m // 4, m % 4
                            _lab(nc.tensor.matmul(
                                pg[:, ty, hc],
                                wih_sb[:, k, 128 * m : 128 * m + 128],
                                x_sb[:, k, 2 * g : 2 * g + 2, :],
                                start=False,
                                stop=False,
                            ), f"p1x g{g} k{k} m{m}")

                def emit_phase3(g):
                    # out granule: steps 4g..4g+3 (hh slots 4g+1..4g+4);
                    # emitted 2 granules late so all operands are ready
                    # (pure cushion work for the PE queue).
                    po = ps3_p.tile([128, OCH, 4, BL], FP32, tag="po", name=f"po{g}")
                    for oc in range(OCH):
                        for k in range(KT):
                            hh = (hhA, hhB)[k // 2]
                            _lab(nc.tensor.matmul(
                                po[:, oc],
                                wlin_sb[:, k, 128 * oc : 128 * oc + 128],
                                hh[:, k % 2, 4 * g + 1 : 4 * g + 5, :],
                                start=(k == 0),
                                stop=(k == KT - 1),
                            ), f"p3 g{g} oc{oc} k{k}")
                    st = stag_p.tile([128, OCH, 4, BL], FP32, tag="st", name=f"st{g}")
                    nc.gpsimd.tensor_copy(st[:], po[:])
                    nc.sync.dma_start(outp[:, :, 4 * g : 4 * g + 4, :], st[:])

                # lookahead of 1 granule: the pg slot's previous reader is
                # then 4+ steps old, so the WAR wait on the bias matmul is
                # long-satisfied and never blocks the PE queue.
                LA_G = 1
                for g in range(LA_G):
                    emit_phase1(g, 0)
                    emit_phase1(g, 1)

                for t in range(T):
                    # cushion: half a phase-1 granule of ready matmuls ahead
                    # of the blocked W_hh block
                    g, khalf = (t + 2 * LA_G) // 2, t % 2
                    if g < T // 2:
                        emit_phase1(g, khalf)
                    if t % 4 == 0 and t >= 8:
                        emit_phase3(t // 4 - 2)
                    pg = pg_tiles[t // 2]
                    # W_hh matmuls, k-major so k=0,1 (needing only the
                    # A-half of h(t-1)) issue while the B-half finishes.
                    # Within each k: A-half gate chunks first.
                    for k in range(KT):
                        hh = (hhA, hhB)[k // 2]
                        rhs = hh[:, k % 2, t, :]
                        for m in (0, 1, 4, 5, 8, 9, 12, 13, 2, 3, 6, 7, 10, 11, 14, 15):
                            ty, hc = m // 4, m % 4
                            _lab(nc.tensor.matmul(
                                pg[:, ty, hc, t % 2],
                                whh_sb[:, k, 128 * m : 128 * m + 128],
                                rhs,
                                start=False,
                                stop=(k == KT - 1),
                            ), f"whh t{t} k{k} m{m}")

                    ts = t % 2
                    acts = acts_p.tile([128, 4, KT, BL], BF16, tag="acts", name="acts")
                    fc = tmps_p.tile([128, KT, BL], FP32, tag="fc", name="fc")
                    ig = tmps_p.tile([128, KT, BL], FP32, tag="ig", name="ig")
                    tct = tmps_p.tile([128, KT, BL], BF16, tag="tct", name="tct")
                    # per half: ONE sigmoid covers all 4 gate types (the g
                    # rows of W/bias are host-scaled by 2: tanh(x) =
                    # 2*sig(2x)-1, the affine fixup folds into the DVE ops:
                    # ig' = (sig_g - 0.5)*i = i*g/2;  c = 2*ig' + f*c).
                    # DVE fc, ig, cadd; ACT tct; POOL hmul.
                    for h2 in range(2):
                        cs = slice(2 * h2, 2 * h2 + 2)
                        # sigmoid split (i,g) then (o,f): the ig product can
                        # start ~300ns earlier
                        _lab(nc.scalar.activation(
                            acts[:, 2:4, cs, :], pg[:, 2:4, cs, ts, :], AF.Sigmoid
                        ), f"sigig t{t} h{h2}")
                        _lab(nc.scalar.activation(
                            acts[:, 0:2, cs, :], pg[:, 0:2, cs, ts, :], AF.Sigmoid
                        ), f"sigof t{t} h{h2}")
                        _lab(nc.vector.scalar_tensor_tensor(
                            ig[:, cs, :], acts[:, 3, cs, :], -0.5, acts[:, 2, cs, :],
                            mybir.AluOpType.add, mybir.AluOpType.mult,
                        ), f"ig t{t} h{h2}")
                        _lab(nc.vector.tensor_mul(fc[:, cs, :], acts[:, 1, cs, :], c_st[:, cs, :]), f"fc t{t} h{h2}")
                        _lab(nc.vector.scalar_tensor_tensor(
                            c_st[:, cs, :], ig[:, cs, :], 2.0, fc[:, cs, :],
                            mybir.AluOpType.mult, mybir.AluOpType.add,
                        ), f"cadd t{t} h{h2}")
                        # ACT: tanh(c); DVE: h = o * tanh(c) -> hh slot t+1
                        _lab(nc.scalar.activation(tct[:, cs, :], c_st[:, cs, :], AF.Tanh), f"tct t{t} h{h2}")
                        hh = (hhA, hhB)[h2]
                        _lab(nc.vector.tensor_mul(
                            hh[:, :, t + 1, :], acts[:, 0, cs, :], tct[:, cs, :]
                        ), f"hmul t{t} h{h2}")
                emit_phase3(TCH - 2)
                emit_phase3(TCH - 1)
    nc.compile()
    return nc


@functools.lru_cache(maxsize=1)
def _program():
    return build_nc()


def _gate_perm():
    # PyTorch gate row order: i (0:H), f (H:2H), g (2H:3H), o (3H:4H).
    # Target: 16 chunks of 128, chunk m=(type, hc) with type order
    # [o f i g]; within a type block the hidden units are in natural
    # order (hc-major, 128 each).
    off = {0: 3 * H, 1: 1 * H, 2: 0 * H, 3: 2 * H}  # o, f, i, g
    perm = []
    for m in range(NCH):
        ty, hc = m // 4, m % 4
        perm += list(range(off[ty] + 128 * hc, off[ty] + 128 * hc + 128))
    return np.asarray(perm)


def _prep_core(x, W_ih, W_hh, b_ih, b_hh, W_lin, direction, bs):
    perm = _gate_perm()
    bf16 = ml_dtypes.bfloat16
    xs = np.asarray(x)[:, bs : bs + BL, :]
    if direction == 1:
        xs = xs[::-1]
    # xT[p, k, t, b] = xs[t, b, 128k+p]
    xT = np.ascontiguousarray(
        xs.reshape(T, BL, KT, 128).transpose(3, 2, 0, 1)
    ).astype(bf16)
    Wp_ih = np.asarray(W_ih)[perm].copy()  # [G4, IN]
    Wp_hh = np.asarray(W_hh)[perm].copy()
    bp = (np.asarray(b_ih) + np.asarray(b_hh))[perm].astype(np.float32)
    # g gate (type block 3) scaled by 2: tanh(x) = 2*sigmoid(2x) - 1
    Wp_ih[3 * H :] *= 2.0
    Wp_hh[3 * H :] *= 2.0
    bp[3 * H :] *= 2.0
    wih = np.ascontiguousarray(Wp_ih.T.reshape(KT, 128, G4).transpose(1, 0, 2)).astype(
        bf16
    )
    whh = np.ascontiguousarray(Wp_hh.T.reshape(KT, 128, G4).transpose(1, 0, 2)).astype(
        bf16
    )
    Wl = np.asarray(W_lin)[:, direction * H : (direction + 1) * H]  # [OUT, H]
    wlin = np.ascontiguousarray(Wl.T.reshape(KT, 128, OUT).transpose(1, 0, 2)).astype(
        bf16
    )
    return {
        "xT": xT,
        "wih": wih,
        "whh": whh,
        "bias1": bp.astype(bf16).reshape(1, G4),
        "ones1": np.ones((1, BL), dtype=bf16),
        "wlin": wlin,
    }


def run_cores(inputs, trace=False):
    """Build per-core in_maps, run on 8 cores, return BassKernelResults."""
    in_maps = []
    for core in range(NCORES):
        direction = core // 4
        bs = (core % 4) * BL
        wk = "f" if direction == 0 else "b"
        in_maps.append(
            _prep_core(
                inputs["x"],
                inputs[f"W_ih_{wk}"],
                inputs[f"W_hh_{wk}"],
                inputs[f"b_ih_{wk}"],
                inputs[f"b_hh_{wk}"],
                inputs["W_lin"],
                direction,
                bs,
            )
        )
    nc = _program()
    return run_bass_kernel_spmd(nc, in_maps, list(range(NCORES)), trace=trace)


def _assemble(results, b_lin):
    # per-core outp: [128(p), OCH, T, BL]; out[t, b, 128*oc+p] = outp[p, oc, t, b]
    out = np.zeros((T, B, OUT), np.float32)
    for core in range(NCORES):
        direction = core // 4
        bs = (core % 4) * BL
        dev = np.asarray(results[core]["outp"], np.float32)  # [128, OCH, T, BL]
        part = dev.transpose(2, 3, 1, 0).reshape(T, BL, OUT)
        if direction == 1:
            part = part[::-1]
        out[:, bs : bs + BL, :] += part
    out += np.asarray(b_lin, np.float32)[None, None, :]
    return out


def kernel(**inputs):
    res = run_cores(inputs, trace=False)
    return _assemble(res.results, inputs["b_lin"])


# revision 15
# speedup vs baseline: 1.2017x; 1.2017x over previous
"""Bidirectional LSTM Trainium2 Bass kernel (transposed formulation).

Problem: T=128, B=128, IN=512, H=512, OUT=512 (fp32 reference).
Sharding: data-parallel over batch + direction-parallel:
  cores 0-3: forward LSTM, batch slices 0:32, 32:64, 64:96, 96:128
  cores 4-7: backward LSTM (time-reversed x), same batch slices

Transposed layout: gates live on the PARTITION axis (16 stationary
chunks of 128 gates = (type o/f/i/g) x (hidden 128-chunk)), the batch
(32) is the matmul moving dim.  PE matmul cost scales with the moving
free size only, so this quarters TensorE work vs. batch-in-partition:
  - phase 1 (xw = x @ W_ih.T + bias) accumulates straight into the
    per-step PSUM bank: bias seeded by a K=1 ones-matmul (start=True),
    then 4 x-k-tile matmuls, then at step time 4 W_hh k-tile matmuls.
  - per-step PSUM bank: tile [128, 4(type), 4(hid chunk), 32] fp32
    (= 512 cols = exactly one 2KB PSUM bank), 6 banks in flight.
  - h is produced directly in transposed layout [hid-in-chunk(128),
    chunk, t, batch] -> no transpose instructions at all; the next
    step's matmuls and phase 3 read it as the moving operand.
  - activations: sigmoid over (o,f,i) block slices, tanh over g;
    cell update on DVE; h = o*tanh(c) on GPSIMD (idle engine) so the
    DVE queue is never blocked behind ACT.
  - phase 3 (out = h @ W_lin_dir.T) per 4-step granule into 1 PSUM
    bank (double buffered), evacuated by GPSIMD, DMA'd per granule.
Host combines: out = out_fwd + flip_t(out_bwd) + b_lin.

All matmuls bf16 (fp32 PSUM accumulation); cell state stays fp32.
"""

import sys

sys.path.insert(0, "/opt/trn_rl_repo")

import functools
import os

import ml_dtypes
import numpy as np

import concourse.bass as bass
import concourse.tile as tile
from concourse import bacc, mybir
from concourse.bass_utils import run_bass_kernel_spmd

T, B, IN, H, OUT = 128, 128, 512, 512, 512
NCORES = 8
BL = B // 4  # batch per core (4 cores per direction)
G4 = 4 * H  # 2048 gate columns
KT = IN // 128  # 4 k-tiles of 128
NCH = 16  # gate M-chunks: (type o/f/i/g) x (hidden chunk 0..3)
OCH = OUT // 128  # 4 output column chunks
TCH = T // 4  # 32 output granules of 4 timesteps

KNOB_LOOKAHEAD = int(os.environ.get("LSTM_LOOKAHEAD", "4"))
KNOB_PG_BUFS = int(os.environ.get("LSTM_PG_BUFS", "3"))

BF16 = mybir.dt.bfloat16
FP32 = mybir.dt.float32
AF = mybir.ActivationFunctionType

LABELS = {}  # instruction name -> human label (for sim diagnostics)


def _lab(inst, label):
    try:
        LABELS[inst.ins.name] = label
    except AttributeError:
        pass
    return inst


def build_nc(reps=1):
    nc = bacc.Bacc(None, target_bir_lowering=False)
    xT = nc.dram_tensor("xT", [128, KT, T, BL], BF16, kind="ExternalInput")
    wih = nc.dram_tensor("wih", [128, KT, G4], BF16, kind="ExternalInput")
    whh = nc.dram_tensor("whh", [128, KT, G4], BF16, kind="ExternalInput")
    bias1 = nc.dram_tensor("bias1", [1, G4], BF16, kind="ExternalInput")
    ones1 = nc.dram_tensor("ones1", [1, 2 * BL], BF16, kind="ExternalInput")
    wlin = nc.dram_tensor("wlin", [128, KT, OUT], BF16, kind="ExternalInput")
    outp = nc.dram_tensor("outp", [128, OCH, T, BL], FP32, kind="ExternalOutput")

    LA = KNOB_LOOKAHEAD

    with tile.TileContext(nc) as tc:
        with (
            tc.tile_pool(name="const", bufs=1) as constp,
            tc.tile_pool(name="acts", bufs=2) as acts_p,
            tc.tile_pool(name="tmps", bufs=2) as tmps_p,
            tc.tile_pool(name="stag", bufs=2) as stag_p,
            tc.tile_pool(name="pg", bufs=KNOB_PG_BUFS, space="PSUM") as pg_p,
            tc.tile_pool(name="ps3", bufs=2, space="PSUM") as ps3_p,
        ):
            wih_sb = constp.tile([128, KT, G4], BF16)
            nc.sync.dma_start(wih_sb[:], wih[:])
            bias_sb = constp.tile([1, G4], BF16)
            nc.sync.dma_start(bias_sb[:], bias1[:])
            ones_sb = constp.tile([1, 2 * BL], BF16)
            nc.sync.dma_start(ones_sb[:], ones1[:])
            # x in 4 time-quarters so phase 1 can start after the first
            x_sb = constp.tile([128, KT, T, BL], BF16)
            for q in range(4):
                nc.sync.dma_start(
                    x_sb[:, :, 32 * q : 32 * q + 32, :], xT[:, :, 32 * q : 32 * q + 32, :]
                )
            whh_sb = constp.tile([128, KT, G4], BF16)
            nc.sync.dma_start(whh_sb[:], whh[:])
            wlin_sb = constp.tile([128, KT, OUT], BF16)
            nc.sync.dma_start(wlin_sb[:], wlin[:])

            # h history, transposed: hh?[p, c, t+1, b] = h_t[128*(2?+c)+p, b]
            # split into per-half tiles so step t+1's k=0,1 matmuls depend
            # only on the A-half write.
            hhA = constp.tile([128, 2, T + 1, BL], BF16)
            hhB = constp.tile([128, 2, T + 1, BL], BF16)
            # cell state [p, hid chunk, b], fp32
            c_st = constp.tile([128, KT, BL], FP32)

            for _rep in range(reps):
                nc.vector.memset(c_st[:], 0.0)
                nc.vector.memset(hhA[:, :, 0, :], 0.0)
                nc.vector.memset(hhB[:, :, 0, :], 0.0)

                pg_tiles = {}

                def emit_phase1(g, khalf):
                    # 2-step granule, half the k-tiles per call: emitted as a
                    # ready-work cushion in front of each step's W_hh block so
                    # the PE exec queue never starves during the h(t-1) wait.
                    # bias seed: K=1 ones matmul (start=True) in the first half.
                    if khalf == 0:
                        pg = pg_p.tile(
                            [128, 4, KT, 2, BL], FP32, tag="pg", name=f"pg{g}"
                        )
                        pg_tiles[g] = pg
                        for m in range(NCH):
                            ty, hc = m // 4, m % 4
                            _lab(nc.tensor.matmul(
                                pg[:, ty, hc],
                                bias_sb[:, 128 * m : 128 * m + 128],
                                ones_sb[:],
                                start=True,
                                stop=False,
                            ), f"p1bias g{g} m{m}")
                    pg = pg_tiles[g]
                    for k in (khalf * 2, khalf * 2 + 1):
                        for m in range(NCH):
                            ty, hc = m // 4, m % 4
                            _lab(nc.tensor.matmul(
                                pg[:, ty, hc],
                                wih_sb[:, k, 128 * m : 128 * m + 128],
                                x_sb[:, k, 2 * g : 2 * g + 2, :],
                                start=False,
                                stop=False,
                            ), f"p1x g{g} k{k} m{m}")

                def emit_phase3(g):
                    # out granule: steps 4g..4g+3 (hh slots 4g+1..4g+4);
                    # emitted 2 granules late so all operands are ready
                    # (pure cushion work for the PE queue).
                    po = ps3_p.tile([128, OCH, 4, BL], FP32, tag="po", name=f"po{g}")
                    for oc in range(OCH):
                        for k in range(KT):
                            hh = (hhA, hhB)[k // 2]
                            _lab(nc.tensor.matmul(
                                po[:, oc],
                                wlin_sb[:, k, 128 * oc : 128 * oc + 128],
                                hh[:, k % 2, 4 * g + 1 : 4 * g + 5, :],
                                start=(k == 0),
                                stop=(k == KT - 1),
                            ), f"p3 g{g} oc{oc} k{k}")
                    st = stag_p.tile([128, OCH, 4, BL], FP32, tag="st", name=f"st{g}")
                    nc.gpsimd.tensor_copy(st[:], po[:])
                    nc.sync.dma_start(outp[:, :, 4 * g : 4 * g + 4, :], st[:])

                # lookahead of 1 granule: the pg slot's previous reader is
                # then 4+ steps old, so the WAR wait on the bias matmul is
                # long-satisfied and never blocks the PE queue.
                LA_G = 1
                for g in range(LA_G):
                    emit_phase1(g, 0)
                    emit_phase1(g, 1)

                for t in range(T):
                    # cushion: half a phase-1 granule of ready matmuls ahead
                    # of the blocked W_hh block
                    g, khalf = (t + 2 * LA_G) // 2, t % 2
                    if g < T // 2:
                        emit_phase1(g, khalf)
                    if t % 4 == 0 and t >= 8:
                        emit_phase3(t // 4 - 2)
                    pg = pg_tiles[t // 2]
                    # W_hh matmuls, k-major so k=0,1 (needing only the
                    # A-half of h(t-1)) issue while the B-half finishes.
                    # Within each k: A-half gate chunks first.
                    for k in range(KT):
                        hh = (hhA, hhB)[k // 2]
                        rhs = hh[:, k % 2, t, :]
                        for m in (0, 1, 4, 5, 8, 9, 12, 13, 2, 3, 6, 7, 10, 11, 14, 15):
                            ty, hc = m // 4, m % 4
                            _lab(nc.tensor.matmul(
                                pg[:, ty, hc, t % 2],
                                whh_sb[:, k, 128 * m : 128 * m + 128],
                                rhs,
                                start=False,
                                stop=(k == KT - 1),
                            ), f"whh t{t} k{k} m{m}")

                    ts = t % 2
                    acts = acts_p.tile([128, 4, KT, BL], BF16, tag="acts", name="acts")
                    fc = tmps_p.tile([128, KT, BL], FP32, tag="fc", name="fc")
                    ig = tmps_p.tile([128, KT, BL], FP32, tag="ig", name="ig")
                    tct = tmps_p.tile([128, KT, BL], BF16, tag="tct", name="tct")
                    # per half: ONE sigmoid covers all 4 gate types (the g
                    # rows of W/bias are host-scaled by 2: tanh(x) =
                    # 2*sig(2x)-1, the affine fixup folds into the DVE ops:
                    # ig' = (sig_g - 0.5)*i = i*g/2;  c = 2*ig' + f*c).
                    # DVE fc, ig, cadd; ACT tct; POOL hmul.
                    for h2 in range(2):
                        cs = slice(2 * h2, 2 * h2 + 2)
                        _lab(nc.scalar.activation(
                            acts[:, :, cs, :], pg[:, :, cs, ts, :], AF.Sigmoid
                        ), f"sig t{t} h{h2}")
                        _lab(nc.vector.scalar_tensor_tensor(
                            ig[:, cs, :], acts[:, 3, cs, :], -0.5, acts[:, 2, cs, :],
                            mybir.AluOpType.add, mybir.AluOpType.mult,
                        ), f"ig t{t} h{h2}")
                        _lab(nc.vector.tensor_mul(fc[:, cs, :], acts[:, 1, cs, :], c_st[:, cs, :]), f"fc t{t} h{h2}")
                        _lab(nc.vector.scalar_tensor_tensor(
                            c_st[:, cs, :], ig[:, cs, :], 2.0, fc[:, cs, :],
                            mybir.AluOpType.mult, mybir.AluOpType.add,
                        ), f"cadd t{t} h{h2}")
                        # ACT: tanh(c); DVE: h = o * tanh(c) -> hh slot t+1
                        _lab(nc.scalar.activation(tct[:, cs, :], c_st[:, cs, :], AF.Tanh), f"tct t{t} h{h2}")
                        hh = (hhA, hhB)[h2]
                        _lab(nc.vector.tensor_mul(
                            hh[:, :, t + 1, :], acts[:, 0, cs, :], tct[:, cs, :]
                        ), f"hmul t{t} h{h2}")
                emit_phase3(TCH - 2)
                emit_phase3(TCH - 1)
    nc.compile()
    return nc


@functools.lru_cache(maxsize=1)
def _program():
    return build_nc()


def _gate_perm():
    # PyTorch gate row order: i (0:H), f (H:2H), g (2H:3H), o (3H:4H).
    # Target: 16 chunks of 128, chunk m=(type, hc) with type order
    # [o f i g]; within a type block the hidden units are in natural
    # order (hc-major, 128 each).
    off = {0: 3 * H, 1: 1 * H, 2: 0 * H, 3: 2 * H}  # o, f, i, g
    perm = []
    for m in range(NCH):
        ty, hc = m // 4, m % 4
        perm += list(range(off[ty] + 128 * hc, off[ty] + 128 * hc + 128))
    return np.asarray(perm)


def _prep_core(x, W_ih, W_hh, b_ih, b_hh, W_lin, direction, bs):
    perm = _gate_perm()
    bf16 = ml_dtypes.bfloat16
    xs = np.asarray(x)[:, bs : bs + BL, :]
    if direction == 1:
        xs = xs[::-1]
    # xT[p, k, t, b] = xs[t, b, 128k+p]
    xT = np.ascontiguousarray(
        xs.reshape(T, BL, KT, 128).transpose(3, 2, 0, 1)
    ).astype(bf16)
    Wp_ih = np.asarray(W_ih)[perm].copy()  # [G4, IN]
    Wp_hh = np.asarray(W_hh)[perm].copy()
    bp = (np.asarray(b_ih) + np.asarray(b_hh))[perm].astype(np.float32)
    # g gate (type block 3) scaled by 2: tanh(x) = 2*sigmoid(2x) - 1
    Wp_ih[3 * H :] *= 2.0
    Wp_hh[3 * H :] *= 2.0
    bp[3 * H :] *= 2.0
    wih = np.ascontiguousarray(Wp_ih.T.reshape(KT, 128, G4).transpose(1, 0, 2)).astype(
        bf16
    )
    whh = np.ascontiguousarray(Wp_hh.T.reshape(KT, 128, G4).transpose(1, 0, 2)).astype(
        bf16
    )
    Wl = np.asarray(W_lin)[:, direction * H : (direction + 1) * H]  # [OUT, H]
    wlin = np.ascontiguousarray(Wl.T.reshape(KT, 128, OUT).transpose(1, 0, 2)).astype(
        bf16
    )
    return {
        "xT": xT,
        "wih": wih,
        "whh": whh,
        "bias1": bp.astype(bf16).reshape(1, G4),
        "ones1": np.ones((1, BL), dtype=bf16),
        "wlin": wlin,
    }


def run_cores(inputs, trace=False):
    """Build per-core in_maps, run on 8 cores, return BassKernelResults."""
    in_maps = []
    for core in range(NCORES):
        direction = core // 4
        bs = (core % 4) * BL
        wk = "f" if direction == 0 else "b"
        in_maps.append(
            _prep_core(
                inputs["x"],
                inputs[f"W_ih_{wk}"],
                inputs[f"W_hh_{wk}"],
                inputs[f"b_ih_{wk}"],
                inputs[f"b_hh_{wk}"],
                inputs["W_lin"],
                direction,
                bs,
            )
        )
    nc = _program()
    return run_bass_kernel_spmd(nc, in_maps, list(range(NCORES)), trace=trace)


def _assemble(results, b_lin):
    # per-core outp: [128(p), OCH, T, BL]; out[t, b, 128*oc+p] = outp[p, oc, t, b]
    out = np.zeros((T, B, OUT), np.float32)
    for core in range(NCORES):
        direction = core // 4
        bs = (core % 4) * BL
        dev = np.asarray(results[core]["outp"], np.float32)  # [128, OCH, T, BL]
        part = dev.transpose(2, 3, 1, 0).reshape(T, BL, OUT)
        if direction == 1:
            part = part[::-1]
        out[:, bs : bs + BL, :] += part
    out += np.asarray(b_lin, np.float32)[None, None, :]
    return out


def kernel(**inputs):
    res = run_cores(inputs, trace=False)
    return _assemble(res.results, inputs["b_lin"])
